# revision 4
# baseline (speedup 1.0000x reference)
"""MoE positionwise FFN (top-2 of 8 experts) on 8 TRN2 NeuronCores.

Strategy: expert-parallel, host-routed. The router (logits -> top-2 ->
softmax gates) is exact fp32 on host (as is the final scatter-add
combine, matching the reference semantics). Each core owns one expert:
the host gathers that expert's routed tokens into a compact [D, C]
bf16 input, and the device kernel is a pure dense FFN:

    h = relu(w1 @ x + b1)   (D -> F)
    y = w2 @ h + b2         (F -> D)

over C tokens in blocks (first block largest so block-0 compute
consumes the weight stream slower than DMA supplies it), weight-
stationary bf16 matmuls, fp32 PSUM accumulation. Weights stream in
1 MB chunks alternated across BOTH HWDGE rings (sync + scalar) for
full HBM bandwidth; biases are passed pre-transposed [128, n] so
their DMA is contiguous. The PE is pre-warmed so the HAM clock gate
is at 8/8 when the real matmuls start.

Self-contained: hardcodes shapes for B=2,T=2048,D=1024,F=4096,E=8,K=2.
"""
import math

import numpy as np
import ml_dtypes

S = 4096
D = 1024
F = 4096
E = 8
NTD = D // 128   # 8 d-tiles
NTF = F // 128   # 32 f-tiles

_cache: dict = {}
LAST_RES = None


def _plan_blocks(maxcnt: int):
    C = max(192, int(math.ceil(maxcnt / 32)) * 32)
    s0 = min(448, C)
    rem = C - s0
    if rem == 0:
        return (s0,)
    s1 = int(math.ceil(rem / 2 / 32)) * 32
    s2 = rem - s1
    if s2 == 0:
        return (s0, s1)
    return (s0, s1, s2)


def _build(blocks: tuple):
    import concourse.bacc as bacc
    import concourse.tile as tile
    import concourse.mybir as mybir
    from concourse.tile import add_dep_helper

    f32 = mybir.dt.float32
    bf16 = mybir.dt.bfloat16
    ACT = mybir.ActivationFunctionType

    C = sum(blocks)
    offs = [sum(blocks[:i]) for i in range(len(blocks))]

    nc = bacc.Bacc("TRN2", target_bir_lowering=False, debug=False, num_devices=8)

    xgT_d = nc.dram_tensor("xgT", [D, C], bf16, kind="ExternalInput")
    w1_d = nc.dram_tensor("w1t", [D, F], bf16, kind="ExternalInput")
    w2_d = nc.dram_tensor("w2t", [F, D], bf16, kind="ExternalInput")
    b1_d = nc.dram_tensor("b1r", [128, NTF], f32, kind="ExternalInput")
    b2_d = nc.dram_tensor("b2r", [128, NTD], f32, kind="ExternalInput")
    yg_d = nc.dram_tensor("yg", [128, NTD, C], bf16, kind="ExternalOutput")

    with tile.TileContext(nc) as tc:
        with (
            tc.tile_pool(name="wpool", bufs=1) as wpool,
            tc.tile_pool(name="xr", bufs=1) as xr,
            tc.tile_pool(name="small", bufs=1) as small,
            tc.tile_pool(name="hpool", bufs=1) as hpool,
            tc.tile_pool(name="ypool", bufs=2) as ypool,
            tc.tile_pool(name="psH", bufs=3, space="PSUM") as psH,
            tc.tile_pool(name="psY", bufs=2, space="PSUM") as psY,
            tc.tile_pool(name="psW", bufs=1, space="PSUM") as psW,
        ):
            # ---- PE pre-warm: trip the HAM activity window during the
            # initial DMA so real matmuls start at 2.4 GHz.
            junk = small.tile([128, 256], bf16)
            nc.vector.memset(junk[:], 0.0)
            wps = psW.tile([128, 256], f32)
            for _ in range(18):
                nc.tensor.matmul(wps[:], lhsT=junk[:, 0:128], rhs=junk[:],
                                 start=True, stop=True)

            # ---- DMA streams on the two HWDGE rings.
            # ring B (scalar): xg block0 + biases, then odd weight chunks.
            # ring A (sync):   even weight chunks.
            # Then w2 chunks, then the remaining xg blocks; y-out chunks
            # ride the sync ring after everything.
            xg = xr.tile([128, NTD, C], bf16)
            ringA = []  # sync
            ringB = []  # scalar

            def put(ring, fn):
                d = fn()
                if ring:
                    add_dep_helper(d.ins, ring[-1].ins, sync=True,
                                   reason="ring order")
                ring.append(d)
                return d

            put(ringB, lambda: nc.scalar.dma_start(
                out=xg[:, :, 0 : blocks[0]],
                in_=xgT_d[:, 0 : blocks[0]].rearrange("(dt p) s -> p dt s", p=128)))
            b1_sb = small.tile([128, NTF], f32)
            put(ringB, lambda: nc.scalar.dma_start(out=b1_sb[:], in_=b1_d[:, :]))
            b2_sb = small.tile([128, NTD], f32)
            put(ringB, lambda: nc.scalar.dma_start(out=b2_sb[:], in_=b2_d[:, :]))

            w1_sb = wpool.tile([128, NTD, F], bf16)   # w1_sb[p,dt,f] = w1t[dt*128+p, f]
            w2_sb = wpool.tile([128, NTF, D], bf16)   # w2_sb[p,ft,d] = w2t[ft*128+p, d]
            FC = F // 8
            for i in range(8):
                ring = ringA if i % 2 == 0 else ringB
                eng = nc.sync if i % 2 == 0 else nc.scalar
                put(ring, lambda i=i, eng=eng: eng.dma_start(
                    out=w1_sb[:, :, i * FC : (i + 1) * FC],
                    in_=w1_d[:, i * FC : (i + 1) * FC].rearrange(
                        "(dt p) f -> p dt f", p=128)))
            DC = D // 8
            for i in range(8):
                ring = ringA if i % 2 == 0 else ringB
                eng = nc.sync if i % 2 == 0 else nc.scalar
                put(ring, lambda i=i, eng=eng: eng.dma_start(
                    out=w2_sb[:, :, i * DC : (i + 1) * DC],
                    in_=w2_d[:, i * DC : (i + 1) * DC].rearrange(
                        "(ft p) d -> p ft d", p=128)))
            for blk in range(1, len(blocks)):
                ring = ringA if blk % 2 == 1 else ringB
                eng = nc.sync if blk % 2 == 1 else nc.scalar
                put(ring, lambda blk=blk, eng=eng: eng.dma_start(
                    out=xg[:, :, offs[blk] : offs[blk] + blocks[blk]],
                    in_=xgT_d[:, offs[blk] : offs[blk] + blocks[blk]].rearrange(
                        "(dt p) s -> p dt s", p=128)))

            # ---- FFN over token blocks ---------------------------------
            for blk, TB in enumerate(blocks):
                off = offs[blk]
                xg_blk = xg[:, :, off : off + TB]
                h_sb = hpool.tile([128, NTF, max(blocks)], bf16, tag="h")
                for ft in range(NTF):
                    hp = psH.tile([128, TB], f32, tag="hps")
                    for dt in range(NTD):
                        nc.tensor.matmul(
                            hp[:],
                            lhsT=w1_sb[:, dt, ft * 128 : (ft + 1) * 128],
                            rhs=xg_blk[:, dt, :],
                            start=(dt == 0),
                            stop=(dt == NTD - 1),
                        )
                    nc.scalar.activation(out=h_sb[:, ft, 0:TB], in_=hp[:], func=ACT.Relu,
                                         bias=b1_sb[:, ft : ft + 1], scale=1.0)
                y_blk = ypool.tile([128, NTD, TB], bf16, tag="y")
                for dt in range(NTD):
                    yp = psY.tile([128, TB], f32, tag="yps")
                    for ft in range(NTF):
                        nc.tensor.matmul(
                            yp[:],
                            lhsT=w2_sb[:, ft, dt * 128 : (dt + 1) * 128],
                            rhs=h_sb[:, ft, 0:TB],
                            start=(ft == 0),
                            stop=(ft == NTF - 1),
                        )
                    nc.vector.tensor_scalar_add(y_blk[:, dt, :], yp[:], b2_sb[:, dt : dt + 1])
                    nc.sync.dma_start(out=yg_d[:, dt, off : off + TB],
                                      in_=y_blk[:, dt, :])

    nc.compile()
    return nc


def _get_nc(blocks: tuple):
    if blocks not in _cache:
        _cache[blocks] = _build(blocks)
    return _cache[blocks]


def kernel(x, gate_w, w1, b1, w2, b2, k):
    from concourse.bass_utils import run_bass_kernel_spmd

    assert int(k) == 2
    x = np.asarray(x, dtype=np.float32)
    gate_w = np.asarray(gate_w, dtype=np.float32)
    w1 = np.asarray(w1, dtype=np.float32)
    b1 = np.asarray(b1, dtype=np.float32)
    w2 = np.asarray(w2, dtype=np.float32)
    b2 = np.asarray(b2, dtype=np.float32)
    B, T, _ = x.shape
    xf = x.reshape(S, D)

    # Router (exact fp32, matching the reference's top-2 renormalized
    # softmax; gates applied host-side during the merge).
    logits = xf @ gate_w.T
    top2 = np.argpartition(-logits, 2, axis=1)[:, :2]
    topv = np.take_along_axis(logits, top2, axis=1)              # (S, 2)
    ex = np.exp(topv - topv.max(axis=1, keepdims=True))
    gsm = ex / ex.sum(axis=1, keepdims=True)
    gates = np.zeros((S, E), dtype=np.float32)
    np.put_along_axis(gates, top2, gsm.astype(np.float32), axis=1)

    sel = np.zeros((S, E), dtype=bool)
    np.put_along_axis(sel, top2, True, axis=1)
    toks = [np.nonzero(sel[:, e])[0] for e in range(E)]
    maxcnt = max(len(t) for t in toks)

    blocks = _plan_blocks(maxcnt)
    C = sum(blocks)
    nc = _get_nc(blocks)

    xfT16 = np.ascontiguousarray(xf.T).astype(ml_dtypes.bfloat16)  # [D, S]
    in_maps = []
    for c in range(E):
        tp = np.zeros(C, dtype=np.int64)
        tp[: len(toks[c])] = toks[c]
        in_maps.append({
            "xgT": np.ascontiguousarray(xfT16[:, tp]),
            "w1t": np.ascontiguousarray(w1[c].T).astype(ml_dtypes.bfloat16),
            "w2t": np.ascontiguousarray(w2[c].T).astype(ml_dtypes.bfloat16),
            "b1r": np.ascontiguousarray(b1[c].reshape(NTF, 128).T),
            "b2r": np.ascontiguousarray(b2[c].reshape(NTD, 128).T),
        })

    res = run_bass_kernel_spmd(nc, in_maps, core_ids=list(range(8)))
    global LAST_RES
    LAST_RES = res

    out = np.zeros((S, D), dtype=np.float32)
    for c in range(E):
        cnt = len(toks[c])
        yg = np.asarray(res.results[c]["yg"]).astype(np.float32)  # (128, NTD, C)
        yt = yg.transpose(1, 0, 2).reshape(D, C)                  # d = dt*128+p
        out[toks[c]] += yt[:, :cnt].T * gates[toks[c], c][:, None]
    return out.reshape(B, T, D)
